# revision 1
# baseline (speedup 1.0000x reference)
"""EnhancedSTGCN Trainium2 kernel.

Data-parallel over batch N=128 across 8 NeuronCores (16 samples/core).
Per-core Bass/Tile kernel computes the full ST-GCN pipeline:
  data_bn -> 3x [GraphConv -> BN+ReLU -> tconv(9x1) -> BN + residual -> ReLU -> EMA]
  -> global mean pool -> FC.

Activation layout on-chip: [channels(partitions), t*18+v (free)] per sample.
The GraphConv V-contraction runs on the tensor engine via a transposed-chunk
trick: matmul(lhsT=x_chunk[ci,126]) puts (7t,18v) on PSUM partitions, then a
second matmul against block-diagonal I7 (x) A^T applies the adjacency and
transposes back to [co, 126] in one instruction.

Matmul operands are bf16 (fp32 PSUM accumulation); elementwise chain
(BN/residual/ReLU/EMA-scan) runs in fp32. Set MM_MODE="f32" for full fp32.

All constant weights ship as ONE packed DRAM tensor -> one DMA -> one
semaphore (walrus limits sync-waits per instruction; scattered weight DMAs
across HWDGE queues blow that limit).
"""

import sys

import numpy as np

sys.path.insert(0, "/opt/trn_rl_repo")

V = 18
T0 = 256
NS = 16  # samples per core
N_CORES = 8
ALPHA = 0.85
EPS = 1e-5
MM_MODE = "bf16"  # "bf16" | "f32"

# (ci, co, T_in, stride) per block
BLOCKS = [(2, 64, 256, 1), (64, 128, 256, 2), (128, 256, 128, 2)]

_NC_CACHE = {}


def _mm_np_dtype():
    if MM_MODE == "bf16":
        import ml_dtypes
        return ml_dtypes.bfloat16
    return np.float32


def _wlayout():
    """Packed weight layout: list of (name, rows, f32cols, kind) + offsets.

    kind: "mm" (matmul dtype: bf16 pairs packed in f32 container) | "f32".
    """
    per = 2 if MM_MODE == "bf16" else 4  # bytes/elem
    def mmcols(width):  # f32 columns for `width` mm-dtype elements
        return width * per // 4

    entries = [("aexp", 128, mmcols(128), "mm")]
    for l, (ci, co, T, stride) in enumerate(BLOCKS, 1):
        cop = min(co, 128)
        mhc = (co + 127) // 128
        khc = mhc
        entries.append((f"gwT{l}", ci, mmcols(co), "mm"))
        entries.append((f"twT{l}", cop, mmcols(9 * khc * co), "mm"))
        entries.append((f"rwT{l}", ci, mmcols(co), "mm"))
        for nm in ("s1", "b1", "b2c", "b2c015"):
            entries.append((f"{nm}_{l}", cop, mhc, "f32"))
    entries.append(("fcwT", 128, 20, "f32"))
    entries.append(("fcb", 1, 10, "f32"))
    off = 0
    layout = {}
    for name, rows, cols, kind in entries:
        layout[name] = (rows, off, cols, kind)
        off += cols
    return layout, off


def _build_nc(ns=NS):
    import concourse.bass as bass
    import concourse.tile as tile
    from concourse import bacc, mybir
    from contextlib import ExitStack

    F32 = mybir.dt.float32
    MDT = mybir.dt.bfloat16 if MM_MODE == "bf16" else F32
    AF = mybir.ActivationFunctionType
    OP = mybir.AluOpType

    layout, wtot = _wlayout()

    # Bacc (not raw Bass): its compile() runs generate_event_semaphores(),
    # which splits sync waits to <=1 per instruction (TRN2 HW constraint).
    nc = bacc.Bacc("TRN2", target_bir_lowering=False, debug=False)

    # ---- DRAM I/O ----
    # x arrives with data_bn pre-applied (host-side affine fold), in matmul dtype
    x_d = nc.dram_tensor("x", [ns, 2, T0 * V], MDT, kind="ExternalInput")
    wpack_d = nc.dram_tensor("wpack", [128, wtot], F32, kind="ExternalInput")
    out_d = nc.dram_tensor("out", [ns, 10], F32, kind="ExternalOutput")

    with ExitStack() as ctx:
        tc = ctx.enter_context(tile.TileContext(nc))
        wp = ctx.enter_context(tc.tile_pool(name="wp", bufs=1))

        wtile = wp.tile([128, wtot], F32)
        nc.sync.dma_start(wtile[:], wpack_d[:])

        def wview(name):
            rows, off, cols, kind = layout[name]
            v = wtile[0:rows, off: off + cols]
            if kind == "mm" and MM_MODE == "bf16":
                v = v.bitcast(MDT)
            return v

        aexp = wview("aexp")
        gw_s = {l: wview(f"gwT{l}") for l in (1, 2, 3)}
        tw_s = {l: wview(f"twT{l}") for l in (1, 2, 3)}
        rw_s = {l: wview(f"rwT{l}") for l in (1, 2, 3)}
        s1_s = {l: wview(f"s1_{l}") for l in (1, 2, 3)}
        b1_s = {l: wview(f"b1_{l}") for l in (1, 2, 3)}
        b2c_s = {l: wview(f"b2c_{l}") for l in (1, 2, 3)}
        b2c015_s = {l: wview(f"b2c015_{l}") for l in (1, 2, 3)}
        fcw_s = wview("fcwT")
        fcb_s = wview("fcb")

        alpha_t = wp.tile([128, T0], F32)
        nc.vector.memset(alpha_t[:], ALPHA)
        ones_t = wp.tile([1, ns], F32)
        nc.vector.memset(ones_t[:], 1.0)
        pooled = wp.tile([128, 2 * ns], F32)

        def chunk_list(total, step):
            full, rem = divmod(total, step)
            out = [(i * step, step) for i in range(full)]
            if rem:
                out.append((full * step, rem))
            return out

        with (
            tc.tile_pool(name="xp", bufs=2) as xp,
            tc.tile_pool(name="actp", bufs=1) as actp,
            tc.tile_pool(name="outp", bufs=2) as outp,
            tc.tile_pool(name="smp", bufs=3) as smp,
            tc.tile_pool(name="aps", bufs=2, space="PSUM") as aps,
            tc.tile_pool(name="bps", bufs=2, space="PSUM") as bps,
        ):
            def block(n, l, xin, ci, co, T, stride):
                Tp = T // stride
                mhc = (co + 127) // 128
                khc = mhc
                cop = min(co, 128)
                pdw = (T + 8) * V

                pd = actp.tile([cop, mhc * pdw], MDT, tag="pd", name=f"pd{l}_{n}")
                for mh in range(mhc):
                    nc.gpsimd.memset(pd[:, mh * pdw: mh * pdw + 4 * V], 0.0)
                    nc.gpsimd.memset(pd[:, mh * pdw + (T + 4) * V: (mh + 1) * pdw], 0.0)

                # ---- A-phase: graph conv (channel mix + adjacency) ----
                for (t0, tcn) in chunk_list(T, 7):
                    P = tcn * V
                    ps1 = aps.tile([126, co], F32, tag="ps1", name=f"ps1_{l}_{n}_{t0}")
                    nc.tensor.matmul(ps1[0:P, :], xin[:, t0 * V: t0 * V + P],
                                     gw_s[l], start=True, stop=True)
                    y1 = smp.tile([126, co], MDT, tag="y1", name=f"y1_{l}_{n}_{t0}")
                    nc.vector.tensor_copy(y1[0:P, :], ps1[0:P, :])
                    for mh in range(mhc):
                        ps3 = aps.tile([cop, 126], F32, tag="ps3",
                                       name=f"ps3_{l}_{n}_{t0}_{mh}")
                        nc.tensor.matmul(ps3[:, 0:P],
                                         y1[0:P, mh * 128: mh * 128 + cop],
                                         aexp[0:P, 0:P], start=True, stop=True)
                        nc.scalar.activation(
                            pd[:, mh * pdw + (4 + t0) * V: mh * pdw + (4 + t0 + tcn) * V],
                            ps3[:, 0:P], AF.Relu,
                            bias=b1_s[l][:, mh: mh + 1], scale=s1_s[l][:, mh: mh + 1])

                # ---- B-phase: temporal conv + residual, one PSUM accumulation
                # group per chunk (s2 folded into tw host-side); d1 = relu(0.15u+b)
                # comes straight off PSUM via one ACT op.
                us = actp.tile([cop, mhc * Tp * V], F32, tag="us", name=f"us{l}_{n}")
                inits = {}
                xin3 = xin.rearrange("p (t v) -> p t v", v=V)
                for (t0, tcn) in chunk_list(Tp, 28):
                    NC = tcn * V
                    for mh in range(mhc):
                        pstc = bps.tile([cop, 504], F32, tag="pstc", bufs=3,
                                        name=f"pstc{l}_{n}_{t0}_{mh}")
                        rr = xin3[:, t0 * stride: (t0 + tcn - 1) * stride + 1: stride, :]
                        nc.tensor.matmul(pstc[:, 0:NC],
                                         rw_s[l][:, mh * 128: mh * 128 + cop],
                                         rr, start=True, stop=False)
                        nmm = 9 * khc
                        i = 0
                        for k in range(9):
                            for kh in range(khc):
                                pdsec = pd[:, kh * pdw: (kh + 1) * pdw].rearrange(
                                    "p (t v) -> p t v", v=V)
                                rhs = pdsec[:, stride * t0 + k:
                                            stride * t0 + k + (tcn - 1) * stride + 1: stride, :]
                                woff = (k * khc + kh) * co + mh * 128
                                nc.tensor.matmul(pstc[:, 0:NC],
                                                 tw_s[l][:, woff: woff + cop],
                                                 rhs,
                                                 start=False, stop=(i == nmm - 1))
                                i += 1
                        if t0 == 0:
                            init = smp.tile([128, V], F32, tag="init",
                                            name=f"init{l}_{n}_{mh}")
                            nc.scalar.activation(init[0:cop, :], pstc[:, 0:V],
                                                 AF.Relu, bias=b2c_s[l][:, mh: mh + 1])
                            inits[mh] = init
                        nc.scalar.activation(
                            us[:, mh * Tp * V + t0 * V: mh * Tp * V + t0 * V + NC],
                            pstc[:, 0:NC], AF.Relu,
                            bias=b2c015_s[l][:, mh: mh + 1], scale=1.0 - ALPHA)

                # ---- C-phase: EMA smooth (scan over t) ----
                ot = outp.tile([cop, mhc * Tp * V], MDT, tag="out", name=f"out{l}_{n}")
                for mh in range(mhc):
                    init = inits[mh]
                    osec = ot[:, mh * Tp * V: (mh + 1) * Tp * V]
                    nc.vector.tensor_copy(osec[:, 0:V], init[0:cop, :])
                    o3 = osec.rearrange("p (t v) -> p t v", v=V)
                    d3 = us[:, mh * Tp * V: (mh + 1) * Tp * V].rearrange(
                        "p (t v) -> p t v", v=V)
                    for v in range(V):
                        nc.vector.tensor_tensor_scan(
                            o3[:, 1:Tp, v], alpha_t[0:cop, 0:Tp - 1], d3[:, 1:Tp, v],
                            init[0:cop, v: v + 1], OP.mult, OP.add)
                return ot

            for n in range(ns):
                x_sb = xp.tile([2, T0 * V], MDT, tag="x", name=f"x_{n}")
                nc.sync.dma_start(x_sb[:], x_d[n])
                h = x_sb
                for l, (ci, co, T, stride) in enumerate(BLOCKS, 1):
                    h = block(n, l, h, ci, co, T, stride)
                # global mean pool (sum; 1/(64*18) folded into fc weights)
                for mh in range(2):
                    nc.vector.tensor_reduce(
                        pooled[:, mh * ns + n: mh * ns + n + 1],
                        h[:, mh * 64 * V: (mh + 1) * 64 * V],
                        axis=mybir.AxisListType.X, op=OP.add)

        # ---- FC head ----
        with tc.tile_pool(name="fcps", bufs=1, space="PSUM") as fcps, \
             tc.tile_pool(name="fcout", bufs=1) as fcout:
            ps = fcps.tile([ns, 10], F32)
            nc.tensor.matmul(ps[:], pooled[:, 0:ns], fcw_s[:, 0:10],
                             start=True, stop=False)
            nc.tensor.matmul(ps[:], pooled[:, ns: 2 * ns], fcw_s[:, 10:20],
                             start=False, stop=False)
            nc.tensor.matmul(ps[:], ones_t[:], fcb_s[:], start=False, stop=True)
            osb = fcout.tile([ns, 10], F32)
            nc.scalar.copy(osb[:], ps[:])
            nc.sync.dma_start(out_d[:], osb[:])

    nc.compile()
    return nc


def _host_inputs(inputs, ns=NS):
    """Build the single packed weight tensor (replicated across cores)."""
    f32 = np.float32
    mdt = _mm_np_dtype()
    layout, wtot = _wlayout()
    wpack = np.zeros((128, wtot), f32)

    def put(name, arr):
        rows, off, cols, kind = layout[name]
        if kind == "mm":
            arr = np.ascontiguousarray(arr.astype(mdt))
            if MM_MODE == "bf16":
                assert arr.shape[-1] % 2 == 0
                wpack.view(np.uint32)[0:rows, off: off + cols] = arr.view(np.uint32)
                return
        arr = np.ascontiguousarray(arr.astype(f32))
        wpack[0:rows, off: off + cols] = arr

    A = np.asarray(inputs["A"], f32)
    aexp = np.zeros((128, 128), f32)
    for t in range(7):
        aexp[t * V:(t + 1) * V, t * V:(t + 1) * V] = A.T
    put("aexp", aexp)
    for l, (ci, co, T, stride) in enumerate(BLOCKS, 1):
        cop = min(co, 128)
        mhc = (co + 127) // 128
        khc = mhc
        gw = np.asarray(inputs[f"l{l}_gw"], f32)
        tw = np.asarray(inputs[f"l{l}_tw"], f32)
        rw = np.asarray(inputs[f"l{l}_rw"], f32)[:, :, 0, 0]
        g1 = np.asarray(inputs[f"l{l}_bn1g"], f32)
        bb1 = np.asarray(inputs[f"l{l}_bn1b"], f32)
        gb = np.asarray(inputs[f"l{l}_gb"], f32)
        g2 = np.asarray(inputs[f"l{l}_bn2g"], f32)
        bb2 = np.asarray(inputs[f"l{l}_bn2b"], f32)
        tb = np.asarray(inputs[f"l{l}_tb"], f32)
        rb = np.asarray(inputs[f"l{l}_rb"], f32)
        s1 = g1 / np.sqrt(f32(1.0) + f32(EPS))
        b1v = s1 * gb + bb1
        s2 = g2 / np.sqrt(f32(1.0) + f32(EPS))
        b2c = s2 * tb + bb2 + rb
        b2c015 = f32(1.0 - ALPHA) * b2c
        put(f"gwT{l}", gw.T)
        tws = tw * s2[:, None, None, None]  # fold bn2 scale into tconv weights
        twp = np.zeros((cop, 9 * khc * co), f32)
        for k in range(9):
            for kh in range(khc):
                blk = tws[:, kh * 128: kh * 128 + cop, k, 0].T  # [cop, co]
                twp[:, (k * khc + kh) * co:(k * khc + kh + 1) * co] = blk
        put(f"twT{l}", twp)
        put(f"rwT{l}", rw.T)
        for nm, vec in (("s1", s1), ("b1", b1v), ("b2c", b2c), ("b2c015", b2c015)):
            put(f"{nm}_{l}", np.ascontiguousarray(vec.reshape(mhc, cop).T))
    fcw = np.asarray(inputs["fc_w"], f32)  # [10, 256]
    fcwT = fcw.T / f32(64 * V)  # fold mean pool
    put("fcwT", np.concatenate([fcwT[0:128, :], fcwT[128:256, :]], axis=1))
    put("fcb", np.asarray(inputs["fc_b"], f32).reshape(1, 10))
    return {"wpack": wpack}


def _host_x(inputs):
    """Apply data_bn (eval-mode affine, host fold) and cast to matmul dtype."""
    f32 = np.float32
    x = np.asarray(inputs["x"], f32)  # (N, 2, 256, 18)
    s = (np.asarray(inputs["dbn_g"], f32)
         / np.sqrt(f32(1.0) + f32(EPS))).reshape(2, V)
    b = np.asarray(inputs["dbn_b"], f32).reshape(2, V)
    xb = x * s[None, :, None, :] + b[None, :, None, :]
    return np.ascontiguousarray(xb.reshape(x.shape[0], 2, T0 * V)).astype(_mm_np_dtype())


def kernel(**inputs) -> np.ndarray:
    from concourse.bass_utils import run_bass_kernel_spmd

    n_total = np.asarray(inputs["x"]).shape[0]
    ns = n_total // N_CORES
    key = ("nc", ns)
    if key not in _NC_CACHE:
        _NC_CACHE[key] = _build_nc(ns)
    nc = _NC_CACHE[key]

    shared = _host_inputs(inputs, ns)
    xb = _host_x(inputs)
    in_maps = []
    for c in range(N_CORES):
        m = dict(shared)
        m["x"] = np.ascontiguousarray(xb[c * ns:(c + 1) * ns])
        in_maps.append(m)

    res = run_bass_kernel_spmd(nc, in_maps, core_ids=list(range(N_CORES)))
    return np.concatenate([res.results[c]["out"] for c in range(N_CORES)], axis=0)



# revision 9
# speedup vs baseline: 1.8505x; 1.8505x over previous
"""EnhancedSTGCN Trainium2 kernel (v2).

Data-parallel over batch N=128 across 8 NeuronCores (16 samples/core).
Per-core pipeline: data_bn (host-folded) -> 3x [GraphConv -> BN+ReLU ->
tconv(9x1)+residual -> BN -> ReLU -> EMA] -> mean pool -> FC.

v2 structural changes vs v1:
- Block1 (64 ch) processes SAMPLE PAIRS on the 128 partitions via
  block-diagonal weights: halves block1 work on every engine.
- Per-sample residual/graph weights for paired inputs use zero-padded
  row blocks so every matmul keeps base partition 0 (mixing nonzero
  base partitions in one stream wedges the PE).
- B-phase (tconv) runs k-outer over 2 PSUM banks: consecutive matmuls
  share a stationary -> fewer effective LDWEIGHTS stalls.
- Emission is a wavefront over (pair, block): at each step block1(p),
  block2(p-1), block3(p-2) chunk-units are round-robin interleaved so
  the tensor engine always has independent work -> stays at full
  p-state (2.4 GHz) instead of the 1.2 GHz mid state.
- PSUM chunk groups are batched: one [128,<=504] ACT/cast per 4 A-chunks
  instead of per-chunk ops; casts and bias+relu output ops alternate
  between the vector and scalar engines to balance load.
- BN1 scale is folded into the graph weights, (1-ALPHA) into the
  block1/2 tconv+residual weights, so every output op is a 2-op
  (bias-add, max0) that either engine can run.
- Block3's EMA + mean pool collapse into one weighted reduction
  (column sums of the EMA operator), skipping its 36 scans entirely.
"""

import sys

import numpy as np

sys.path.insert(0, "/opt/trn_rl_repo")

V = 18
T0 = 256
NS = 16  # samples per core
N_CORES = 8
ALPHA = 0.85
EPS = 1e-5
MM_MODE = "bf16"

# (ci, co, T_in, stride) per block
BLOCKS = [(2, 64, 256, 1), (64, 128, 256, 2), (128, 256, 128, 2)]

_NC_CACHE = {}


def _mm_np_dtype():
    import ml_dtypes
    return ml_dtypes.bfloat16


def chunk_list(total, step):
    full, rem = divmod(total, step)
    out = [(i * step, step) for i in range(full)]
    if rem:
        out.append((full * step, rem))
    return out


def _wlayout():
    """Packed weight layout: (name, rows, f32cols, kind) + offsets."""
    def mmcols(width):  # f32 columns holding `width` bf16 elements
        return width // 2

    entries = [
        ("aexp", 126, mmcols(126), "mm"),
        ("gw1p", 4, mmcols(128), "mm"),          # pair graph weights (K=4)
        ("rw1p", 4, mmcols(128), "mm"),          # pair residual, (1-a) folded
        ("tw1q", 128, mmcols(9 * 128), "mm"),    # pair tconv, block-diag
        ("gw2q", 128, mmcols(256), "mm"),        # pair graph (block-diag out)
        ("rw2z", 128, mmcols(256), "mm"),        # 2 zero-block variants
        ("tw2", 128, mmcols(9 * 128), "mm"),
        ("gw3", 128, mmcols(256), "mm"),
        ("rw3", 128, mmcols(256), "mm"),         # 2 mh halves
        ("tw3", 128, mmcols(36 * 128), "mm"),    # (k, kh, mh) blocks
        ("b11", 128, 1, "f32"),
        ("b2c1s", 128, 1, "f32"),
        ("b2c1u", 128, 1, "f32"),
        ("b12", 128, 1, "f32"),
        ("b2c2s", 128, 1, "f32"),
        ("b2c2u", 128, 1, "f32"),
        ("b13", 128, 2, "f32"),
        ("b2c3", 128, 2, "f32"),
        ("g3t", 128, 18 * 64, "f32"),
        ("fcwT", 128, 20, "f32"),
        ("fcb", 1, 10, "f32"),
    ]
    off = 0
    layout = {}
    for name, rows, cols, kind in entries:
        layout[name] = (rows, off, cols, kind)
        off += cols
    return layout, off


def _build_nc(ns=NS):
    import concourse.bass as bass
    import concourse.tile as tile
    from concourse import bacc, mybir
    from contextlib import ExitStack

    F32 = mybir.dt.float32
    MDT = mybir.dt.bfloat16
    AF = mybir.ActivationFunctionType
    OP = mybir.AluOpType

    layout, wtot = _wlayout()
    npair = ns // 2
    INV1A = 1.0 / (1.0 - ALPHA)

    nc = bacc.Bacc("TRN2", target_bir_lowering=False, debug=False)

    x_d = nc.dram_tensor("x", [npair, 4, T0 * V], MDT, kind="ExternalInput")
    wpack_d = nc.dram_tensor("wpack", [128, wtot], F32, kind="ExternalInput")
    out_d = nc.dram_tensor("out", [ns, 10], F32, kind="ExternalOutput")

    with ExitStack() as ctx:
        tc = ctx.enter_context(tile.TileContext(nc))
        wp = ctx.enter_context(tc.tile_pool(name="wp", bufs=1))

        wtile = wp.tile([128, wtot], F32)
        nc.sync.dma_start(wtile[:], wpack_d[:])

        def wview(name):
            rows, off, cols, kind = layout[name]
            v = wtile[0:rows, off: off + cols]
            if kind == "mm":
                v = v.bitcast(MDT)
            return v

        aexp = wview("aexp")
        gw1p, rw1p, tw1q = wview("gw1p"), wview("rw1p"), wview("tw1q")
        gw2q, rw2z, tw2 = wview("gw2q"), wview("rw2z"), wview("tw2")
        gw3, rw3, tw3 = wview("gw3"), wview("rw3"), wview("tw3")
        b11, b2c1s, b2c1u = wview("b11"), wview("b2c1s"), wview("b2c1u")
        b12, b2c2s, b2c2u = wview("b12"), wview("b2c2s"), wview("b2c2u")
        b13, b2c3 = wview("b13"), wview("b2c3")
        g3t = wview("g3t")
        fcw_s, fcb_s = wview("fcwT"), wview("fcb")

        alpha_t = wp.tile([128, T0], F32)
        nc.vector.memset(alpha_t[:], ALPHA)
        ones_t = wp.tile([1, ns], F32)
        nc.vector.memset(ones_t[:], 1.0)
        pooled = wp.tile([128, 2 * ns], F32)

        # persistent pd tiles (borders zeroed once)
        pd1 = [wp.tile([128, 264 * V], MDT, name=f"pd1_{i}") for i in range(2)]
        pd2 = [wp.tile([128, 264 * V], MDT, name=f"pd2_{i}") for i in range(3)]
        pd3 = [wp.tile([128, 2 * 136 * V], MDT, name=f"pd3_{i}")
               for i in range(3)]
        for t in pd1 + pd2:
            nc.gpsimd.memset(t[:, 0:4 * V], 0.0)
            nc.gpsimd.memset(t[:, 260 * V:264 * V], 0.0)
        for t in pd3:
            for kh in range(2):
                o = kh * 136 * V
                nc.gpsimd.memset(t[:, o:o + 4 * V], 0.0)
                nc.gpsimd.memset(t[:, o + 132 * V:o + 136 * V], 0.0)

        # engine alternation for psum->sbuf ops
        tick = [0]

        def cast_out(dst, src):
            tick[0] += 1
            if tick[0] % 2:
                nc.vector.tensor_copy(dst, src)
            else:
                nc.scalar.activation(dst, src, AF.Copy)

        def relu_bias(dst, src, bias_ap):
            tick[0] += 1
            if tick[0] % 2:
                nc.scalar.activation(dst, src, AF.Relu, bias=bias_ap)
            else:
                nc.vector.tensor_scalar(dst, src, bias_ap, 0.0, OP.add, OP.max)

        with (
            tc.tile_pool(name="xp", bufs=2) as xp,
            tc.tile_pool(name="y1p", bufs=3) as y1p,
            tc.tile_pool(name="us1p", bufs=2) as us1p,
            tc.tile_pool(name="us2p", bufs=3) as us2p,
            tc.tile_pool(name="us3p", bufs=4) as us3p,
            tc.tile_pool(name="ot1p", bufs=2) as ot1p,
            tc.tile_pool(name="ot2p", bufs=4) as ot2p,
            tc.tile_pool(name="scrp", bufs=1) as scrp,
            tc.tile_pool(name="ps1", bufs=2, space="PSUM") as ps1p,
            tc.tile_pool(name="psB", bufs=2, space="PSUM") as psBp,
            tc.tile_pool(name="pstc", bufs=2, space="PSUM") as pstcp,
        ):
            ACH12 = chunk_list(T0, 7)   # A-phase chunks, blocks 1/2 (37)
            ACH3 = chunk_list(128, 7)   # block 3 (19)

            def a_phase(l, u, src, pds):
                """GraphConv + BN1 + ReLU -> pd.

                l: block idx (1..3); u: unit tag; src: lhsT source tile
                (b1: xpair[4, TV]; b2: ot1 pair [128, TV]; b3: ot2 [128, TV2]).
                pds: list of target pd tiles (b1: [pdpair]; b2: [pd_s0, pd_s1]
                ordered by y1 column block; b3: [pd3, pd3] with kh offsets).
                """
                if l == 1:
                    rhsw, N1, cpb, bias, ACH = gw1p, 128, 4, b11, ACH12
                elif l == 2:
                    rhsw, N1, cpb, bias, ACH = gw2q, 256, 2, b12, ACH12
                else:
                    rhsw, N1, cpb, bias, ACH = gw3, 256, 2, b13, ACH3
                ntgt = len(pds)
                nch = len(ACH)
                ps1 = None
                y1 = None
                psB = [None] * ntgt
                pend = []  # (tgt, psB, colw, pd_off) awaiting output op
                for c, (t0, tcn) in enumerate(ACH):
                    P = tcn * V
                    ci = c % cpb
                    if ci == 0:
                        ps1 = ps1p.tile([126, 512], F32, tag="ps1",
                                        name=f"ps1_{l}_{u}_{c}")
                    # mm1: transpose-chunk channel mix
                    nc.tensor.matmul(ps1[0:P, ci * N1:(ci + 1) * N1],
                                     src[:, t0 * V: t0 * V + P], rhsw,
                                     start=True, stop=True)
                    if ci == cpb - 1 or c == nch - 1:
                        y1 = y1p.tile([126, 512], MDT, tag="y1",
                                      name=f"y1_{l}_{u}_{c}")
                        Pc = 126 if ci == cpb - 1 else P
                        cast_out(y1[0:Pc, 0:(ci + 1) * N1],
                                 ps1[0:Pc, 0:(ci + 1) * N1])
                        # ps3 for the chunks in this bank
                        for cc in range(c - ci, c + 1):
                            tt0, ttcn = ACH[cc]
                            PP = ttcn * V
                            c4 = cc % 4
                            for tgt in range(ntgt):
                                if c4 == 0:
                                    psB[tgt] = psBp.tile(
                                        [128, 504], F32, tag=f"psB{tgt}",
                                        name=f"psB_{l}_{u}_{cc}_{tgt}")
                                yoff = (cc % cpb) * N1 + tgt * 128
                                nc.tensor.matmul(
                                    psB[tgt][:, c4 * 126: c4 * 126 + PP],
                                    y1[0:PP, yoff: yoff + 128],
                                    aexp[0:PP, 0:PP], start=True, stop=True)
                                if c4 == 3 or cc == nch - 1:
                                    colw = c4 * 126 + PP
                                    base = (cc - c4) * 7
                                    if l == 3:
                                        pdo = tgt * 136 * V + (4 + base) * V
                                        pdt = pds[0]
                                    else:
                                        pdo = (4 + base) * V
                                        pdt = pds[tgt]
                                    bias_ap = bias[:, tgt: tgt + 1] \
                                        if l == 3 else bias[:, 0:1]
                                    relu_bias(pdt[:, pdo: pdo + colw],
                                              psB[tgt][:, 0:colw], bias_ap)
                        yield

            def b_phase(l, u, pdt, rhs_res, us, ot):
                """tconv + residual (PSUM accum) -> bias+relu -> us; init col
                of ot for blocks 1/2; yields per chunk-group."""
                if l == 1:
                    Tp, stride, tws, nk, khc = 256, 1, tw1q, 9, 1
                    b2cs, b2cu = b2c1s, b2c1u
                elif l == 2:
                    Tp, stride, tws, nk, khc = 128, 2, tw2, 9, 1
                    b2cs, b2cu = b2c2s, b2c2u
                else:
                    Tp, stride, tws, nk, khc = 64, 2, tw3, 9, 2
                    b2cs, b2cu = None, None
                mh = u[2] if l == 3 else 0
                BCH = chunk_list(Tp, 28)
                ngrp = (len(BCH) + 1) // 2
                for g in range(ngrp):
                    chs = BCH[2 * g: 2 * g + 2]
                    banks = []
                    for (t0, tcn) in chs:
                        banks.append(pstcp.tile(
                            [128, 504], F32, tag="pstc",
                            name=f"pstc_{l}_{u}_{t0}"))
                    # residual first (start=True)
                    for bi, (t0, tcn) in enumerate(chs):
                        NCc = tcn * V
                        rr = rhs_res[0][:, t0 * stride:
                                        (t0 + tcn - 1) * stride + 1: stride, :]
                        nc.tensor.matmul(banks[bi][:, 0:NCc], rhs_res[1], rr,
                                         start=True, stop=False)
                    # taps, k-outer for stationary reuse
                    i = 0
                    nmm = nk * khc
                    for k in range(nk):
                        for kh in range(khc):
                            if l == 3:
                                woff = ((k * khc + kh) * 2 + mh) * 128
                                pdsec = pdt[:, kh * 136 * V:(kh + 1) * 136 * V]
                            else:
                                woff = k * 128
                                pdsec = pdt[:]
                            pd3v = pdsec.rearrange("p (t v) -> p t v", v=V)
                            last = (i == nmm - 1)
                            for bi, (t0, tcn) in enumerate(chs):
                                NCc = tcn * V
                                rhs = pd3v[:, stride * t0 + k:
                                           stride * t0 + k +
                                           (tcn - 1) * stride + 1: stride, :]
                                nc.tensor.matmul(banks[bi][:, 0:NCc],
                                                 tws[:, woff: woff + 128], rhs,
                                                 start=False, stop=last)
                            i += 1
                    for bi, (t0, tcn) in enumerate(chs):
                        NCc = tcn * V
                        if l == 3:
                            relu_bias(us[:, t0 * V: t0 * V + NCc],
                                      banks[bi][:, 0:NCc],
                                      b2c3[:, mh: mh + 1])
                        else:
                            if t0 == 0:
                                nc.scalar.activation(
                                    ot[:, 0:V], banks[bi][:, 0:V], AF.Relu,
                                    bias=b2cu[:, 0:1], scale=INV1A)
                            relu_bias(us[:, t0 * V: t0 * V + NCc],
                                      banks[bi][:, 0:NCc], b2cs[:, 0:1])
                    yield

            def ema(l, us, ot):
                Tp = 256 if l == 1 else 128
                us3 = us.rearrange("p (t v) -> p t v", v=V)
                ot3 = ot.rearrange("p (t v) -> p t v", v=V)
                for v in range(V):
                    nc.vector.tensor_tensor_scan(
                        ot3[:, 1:Tp, v], alpha_t[:, 0:Tp - 1],
                        us3[:, 1:Tp, v], ot[:, v: v + 1],
                        OP.mult, OP.add)
                    if v == 8:
                        yield
                yield

            def b1_gen(p):
                xpair = xp.tile([4, T0 * V], MDT, tag="x", name=f"x_{p}")
                nc.sync.dma_start(xpair[:], x_d[p])
                pdt = pd1[p % 2]
                yield from a_phase(1, p, xpair, [pdt])
                us = us1p.tile([128, T0 * V], MDT, tag="us1", name=f"us1_{p}")
                ot = ot1p.tile([128, T0 * V], MDT, tag="ot1", name=f"ot1_{p}")
                x3 = xpair.rearrange("p (t v) -> p t v", v=V)
                yield from b_phase(1, p, pdt, (x3, rw1p), us, ot)
                yield from ema(1, us, ot)
                b1_out[p] = ot

            def b2_gen(p):
                ot1 = b1_out[p]
                pda = pd2[(2 * p) % 3]
                pdb = pd2[(2 * p + 1) % 3]
                yield from a_phase(2, p, ot1, [pda, pdb])
                ot1_3 = ot1.rearrange("p (t v) -> p t v", v=V)
                for s in range(2):
                    pdt = pda if s == 0 else pdb
                    us = us2p.tile([128, 128 * V], MDT, tag="us2",
                                   name=f"us2_{p}_{s}")
                    ot = ot2p.tile([128, 128 * V], MDT, tag="ot2",
                                   name=f"ot2_{p}_{s}")
                    rwz = rw2z[:, s * 128:(s + 1) * 128]
                    yield from b_phase(2, (p, s), pdt, (ot1_3, rwz), us, ot)
                    yield from ema(2, us, ot)
                    b2_out[2 * p + s] = ot

            def b3_gen(s):
                ot2 = b2_out[s]
                pdt = pd3[s % 3]
                yield from a_phase(3, s, ot2, [pdt, pdt])
                ot2_3 = ot2.rearrange("p (t v) -> p t v", v=V)
                for mh in range(2):
                    us = us3p.tile([128, 64 * V], MDT, tag="us3",
                                   name=f"us3_{s}_{mh}")
                    rwm = rw3[:, mh * 128:(mh + 1) * 128]
                    yield from b_phase(3, (s, 0, mh), pdt, (ot2_3, rwm),
                                       us, None)
                    scr = scrp.tile([128, 64 * V], F32, tag="scr",
                                    name=f"scr_{s}_{mh}")
                    nc.vector.tensor_mul(scr[:], us[:], g3t[:])
                    nc.vector.tensor_reduce(
                        pooled[:, mh * ns + s: mh * ns + s + 1], scr[:],
                        axis=mybir.AxisListType.X, op=OP.add)
                    yield

            b1_out = {}
            b2_out = {}
            nsteps = npair + 2
            for step in range(nsteps):
                gens = []
                if step < npair:
                    gens.append(b1_gen(step))
                if 1 <= step <= npair:
                    gens.append(b2_gen(step - 1))
                if step >= 2:
                    gens.append(b3_gen(2 * (step - 2)))
                    gens.append(b3_gen(2 * (step - 2) + 1))
                while gens:
                    nxt = []
                    for g in gens:
                        try:
                            next(g)
                            nxt.append(g)
                        except StopIteration:
                            pass
                    gens = nxt

        # ---- FC head ----
        with tc.tile_pool(name="fcps", bufs=1, space="PSUM") as fcps, \
             tc.tile_pool(name="fcout", bufs=1) as fcout:
            ps = fcps.tile([ns, 10], F32)
            nc.tensor.matmul(ps[:], pooled[:, 0:ns], fcw_s[:, 0:10],
                             start=True, stop=False)
            nc.tensor.matmul(ps[:], pooled[:, ns: 2 * ns], fcw_s[:, 10:20],
                             start=False, stop=False)
            nc.tensor.matmul(ps[:], ones_t[:], fcb_s[:], start=False, stop=True)
            osb = fcout.tile([ns, 10], F32)
            nc.scalar.copy(osb[:], ps[:])
            nc.sync.dma_start(out_d[:], osb[:])

    nc.compile()
    return nc


def _host_inputs(inputs, ns=NS):
    """Build the packed weight tensor (replicated across cores)."""
    f32 = np.float32
    mdt = _mm_np_dtype()
    layout, wtot = _wlayout()
    wpack = np.zeros((128, wtot), f32)

    def put(name, arr):
        rows, off, cols, kind = layout[name]
        if kind == "mm":
            arr = np.ascontiguousarray(arr.astype(mdt))
            assert arr.shape == (rows, cols * 2), (name, arr.shape)
            wpack.view(np.uint32)[0:rows, off: off + cols] = arr.view(np.uint32)
            return
        arr = np.ascontiguousarray(arr.astype(f32))
        assert arr.shape == (rows, cols), (name, arr.shape)
        wpack[0:rows, off: off + cols] = arr

    A = np.asarray(inputs["A"], f32)
    put("aexp", np.kron(np.eye(7, dtype=f32), A.T))

    oma = f32(1.0 - ALPHA)
    sc = {}
    for l, (ci, co, T, stride) in enumerate(BLOCKS, 1):
        g1 = np.asarray(inputs[f"l{l}_bn1g"], f32)
        bb1 = np.asarray(inputs[f"l{l}_bn1b"], f32)
        gb = np.asarray(inputs[f"l{l}_gb"], f32)
        g2 = np.asarray(inputs[f"l{l}_bn2g"], f32)
        bb2 = np.asarray(inputs[f"l{l}_bn2b"], f32)
        tb = np.asarray(inputs[f"l{l}_tb"], f32)
        rb = np.asarray(inputs[f"l{l}_rb"], f32)
        s1 = g1 / np.sqrt(f32(1.0) + f32(EPS))
        s2 = g2 / np.sqrt(f32(1.0) + f32(EPS))
        sc[l] = dict(
            s1=s1, s2=s2,
            b1v=s1 * gb + bb1,
            b2c=s2 * tb + bb2 + rb,
            gw=np.asarray(inputs[f"l{l}_gw"], f32) * s1[:, None],
            tw=np.asarray(inputs[f"l{l}_tw"], f32)[:, :, :, 0]
            * s2[:, None, None],
            rw=np.asarray(inputs[f"l{l}_rw"], f32)[:, :, 0, 0],
        )

    # block1 (pairs; (1-a) folded into tconv/residual/bias)
    c1 = sc[1]
    gw1 = c1["gw"]  # [64, 2], s1-scaled
    gw1p = np.zeros((4, 128), f32)
    for si in range(2):
        gw1p[si * 2: si * 2 + 2, si * 64:(si + 1) * 64] = gw1.T
    put("gw1p", gw1p)
    rw1 = c1["rw"] * oma  # [64, 2]
    rw1p = np.zeros((4, 128), f32)
    for si in range(2):
        rw1p[si * 2: si * 2 + 2, si * 64:(si + 1) * 64] = rw1.T
    put("rw1p", rw1p)
    tw1 = c1["tw"] * oma  # [64, 64, 9]
    tw1q = np.zeros((128, 9 * 128), f32)
    for k in range(9):
        blk = tw1[:, :, k].T  # [ci, co]
        tw1q[0:64, k * 128: k * 128 + 64] = blk
        tw1q[64:128, k * 128 + 64: k * 128 + 128] = blk
    put("tw1q", tw1q)
    put("b11", np.tile(c1["b1v"], 2).reshape(128, 1))
    put("b2c1s", np.tile(c1["b2c"] * oma, 2).reshape(128, 1))
    put("b2c1u", np.tile(c1["b2c"], 2).reshape(128, 1))

    # block2
    c2 = sc[2]
    gw2 = c2["gw"]  # [128, 64]
    gw2q = np.zeros((128, 256), f32)
    gw2q[0:64, 0:128] = gw2.T
    gw2q[64:128, 128:256] = gw2.T
    put("gw2q", gw2q)
    rw2 = c2["rw"] * oma  # [128, 64]
    rw2z = np.zeros((128, 256), f32)
    rw2z[0:64, 0:128] = rw2.T
    rw2z[64:128, 128:256] = rw2.T
    put("rw2z", rw2z)
    tw2 = c2["tw"] * oma  # [128, 128, 9]
    tw2q = np.zeros((128, 9 * 128), f32)
    for k in range(9):
        tw2q[:, k * 128:(k + 1) * 128] = tw2[:, :, k].T
    put("tw2", tw2q)
    put("b12", c2["b1v"].reshape(128, 1))
    put("b2c2s", (c2["b2c"] * oma).reshape(128, 1))
    put("b2c2u", c2["b2c"].reshape(128, 1))

    # block3 (no (1-a) folding; EMA+pool folded into g3t)
    c3 = sc[3]
    gw3 = c3["gw"]  # [256, 128]
    put("gw3", gw3.T)  # [128, 256]
    rw3 = c3["rw"]  # [256, 128]
    rw3q = np.zeros((128, 256), f32)
    rw3q[:, 0:128] = rw3[0:128].T
    rw3q[:, 128:256] = rw3[128:256].T
    put("rw3", rw3q)
    tw3 = c3["tw"]  # [256, 256, 9], s2-scaled
    tw3q = np.zeros((128, 36 * 128), f32)
    for k in range(9):
        for kh in range(2):
            for mh in range(2):
                blk = tw3[mh * 128:(mh + 1) * 128,
                          kh * 128:(kh + 1) * 128, k].T  # [ci, co]
                woff = ((k * 2 + kh) * 2 + mh) * 128
                tw3q[:, woff: woff + 128] = blk
    put("tw3", tw3q)
    put("b13", np.stack([c3["b1v"][0:128], c3["b1v"][128:256]], axis=1))
    put("b2c3", np.stack([c3["b2c"][0:128], c3["b2c"][128:256]], axis=1))
    # EMA column sums: g3[0] = sum a^t; g3[s>=1] = 1 - a^(T-s)
    Tp3 = 64
    a = f32(ALPHA)
    g3 = np.empty(Tp3, f32)
    g3[0] = (1 - a ** Tp3) / (1 - a)
    for s in range(1, Tp3):
        g3[s] = 1 - a ** (Tp3 - s)
    g3t = np.repeat(g3, V)[None, :].repeat(128, axis=0)
    put("g3t", g3t)

    fcw = np.asarray(inputs["fc_w"], f32)  # [10, 256]
    fcwT = fcw.T / f32(Tp3 * V)
    put("fcwT", np.concatenate([fcwT[0:128, :], fcwT[128:256, :]], axis=1))
    put("fcb", np.asarray(inputs["fc_b"], f32).reshape(1, 10))
    return {"wpack": wpack}


def _host_x(inputs):
    """data_bn fold + pair layout: [npairs, 4, T0*V] bf16."""
    f32 = np.float32
    x = np.asarray(inputs["x"], f32)  # (N, 2, 256, 18)
    N = x.shape[0]
    s = (np.asarray(inputs["dbn_g"], f32)
         / np.sqrt(f32(1.0) + f32(EPS))).reshape(2, V)
    b = np.asarray(inputs["dbn_b"], f32).reshape(2, V)
    xb = x * s[None, :, None, :] + b[None, :, None, :]
    xb = xb.reshape(N // 2, 2, 2, T0 * V).reshape(N // 2, 4, T0 * V)
    return np.ascontiguousarray(xb).astype(_mm_np_dtype())


def kernel(**inputs) -> np.ndarray:
    from concourse.bass_utils import run_bass_kernel_spmd

    n_total = np.asarray(inputs["x"]).shape[0]
    ns = n_total // N_CORES
    key = ("nc", ns)
    if key not in _NC_CACHE:
        _NC_CACHE[key] = _build_nc(ns)
    nc = _NC_CACHE[key]

    shared = _host_inputs(inputs, ns)
    xb = _host_x(inputs)
    npair = ns // 2
    in_maps = []
    for c in range(N_CORES):
        m = dict(shared)
        m["x"] = np.ascontiguousarray(xb[c * npair:(c + 1) * npair])
        in_maps.append(m)

    res = run_bass_kernel_spmd(nc, in_maps, core_ids=list(range(N_CORES)))
    return np.concatenate([res.results[c]["out"] for c in range(N_CORES)], axis=0)
